# revision 11
# baseline (speedup 1.0000x reference)
"""Distributed Trainium2 kernel for pre-LN multi-head self-attention.

Reference computation (n=2048, d=1024, 16 heads x 64):
    xn  = LayerNorm(x) * ln_scale + ln_bias
    qkv = xn @ w_qkv ; split -> q,k,v [16, 2048, 64]
    sim = (q @ k^T) * d**-0.5 ; attn = softmax(sim)
    out = concat_heads(attn @ v) @ w_out + b_out

Sharding: 2 heads per core (tensor parallel). Each core:
  - projects its 2 heads' q/k/v, attention in transposed layout,
    ONE AllGather per row chunk, 128-col slice of the final projection.
Host assembles the 8 [128, 2048] outT shards into the [2048, 1024] output.

v3 changes vs v2 (224.6us):
  - RAW x is transposed by the DMA XBAR (dma_start_transpose, 14ns per
    16x128 tile) straight into xT sbuf: no PE transposes, no PSUM
    evacuations, and no LayerNorm->transpose serialization.  LN is
    algebraically folded into the QKV projection instead:
        qT = rstd_row * (W'^T x^T  - csum (x) mu_row + b (x) std_row)
    where W' = ln_scale-scaled weights, csum = column sums of W' and
    b = ln_bias @ W are computed on host and shipped as a tiny [2, 384]
    tensor; the two rank-1 terms are ONE extra [2]-contraction matmul
    accumulated into the QKV PSUM chain; the per-row rstd multiply is
    the PSUM evacuation (gpsimd tensor_mul with a broadcast rstd tile).
    mu/std/rstd rows are built from bn_stats via a tiny PE transpose of
    the per-tile stat columns.  Pre-attention drops ~71us -> ~30us.
  - output projections interleaved into the AllGather chain: po2 half
    idx%2 is free between norm_tail(idx+2) and av(idx+4), so proj(idx)
    runs there (its gather DMA pre-issued one stage earlier, right
    after norm_tail(idx+1)).  Only AG4+proj4 remain in the drain.
"""

import sys

import ml_dtypes
import numpy as np

for _p in ("/opt/trn_rl_repo", "/root/.axon_site/_ro/trn_rl_repo"):
    if _p not in sys.path:
        sys.path.append(_p)

N = 2048          # sequence length
D = 1024          # model dim
HEADS = 16
DH = 64
NCORES = 8
HL = HEADS // NCORES          # heads per core (2)
HC = HL * DH                  # head cols per core (128)
LN_EPS = 1e-6
SIM_SCALE = float(D) ** -0.5  # reference scales by input dim

P = 128
RT = N // P        # 16 row tiles
DC = D // P        # 8 dim chunks
RC_W = 512         # row-chunk width for attention/collective pipeline

MM_DT = "bf16"

# DVE-side fast-exp (Schraudolph bf16 bit trick) for these key chunks of
# every stage; offloads the ACT engine (the attention-phase bottleneck) at
# ~3% max per-element error on the affected attention weights.
DVE_EXP_KCS = (3, 7, 11, 15)
_SCH_A = 128.0 / float(np.log(2.0))          # per (sim*SIM_SCALE) logit
_SCH_B = 127.0 * 128.0 - 5.5                 # exponent bias - centering

_BUILT = None


def _build():
    """Build the SPMD Bass graph (same graph on all 8 cores)."""
    from contextlib import ExitStack

    import concourse.tile as tile
    from concourse import bacc, mybir
    from concourse.masks import make_identity

    f32 = mybir.dt.float32
    dt_mm = {"f32": f32, "f32r": mybir.dt.float32r,
             "bf16": mybir.dt.bfloat16}[MM_DT]
    AF = mybir.ActivationFunctionType

    nc = bacc.Bacc(None, num_devices=NCORES)

    x_d = nc.declare_dram_parameter("x", [N, D], dt_mm, isOutput=False)
    w4_d = nc.declare_dram_parameter("w4", [D, 4 * HC], dt_mm, isOutput=False)
    r2_d = nc.declare_dram_parameter("r2", [2, 3 * HC], dt_mm, isOutput=False)
    bo_d = nc.declare_dram_parameter("bo", [HC], f32, isOutput=False)
    out_d = nc.declare_dram_parameter("out", [HC, N], f32, isOutput=True)

    groups = [list(range(NCORES))]
    # the AllGather chain is saturated end-to-end, so the tail is set by the
    # LAST ops' durations: keep the early chunks big (fewer fixed costs)
    # and the trailing chunks small (short final ops)
    chunks = [(0, 512), (512, 512), (1024, 512), (1536, 256), (1792, 256)]
    S = len(chunks)

    with ExitStack() as ctx:
        tc = ctx.enter_context(tile.TileContext(nc))

        dram = ctx.enter_context(tc.tile_pool(name="dram", bufs=1, space="DRAM"))
        ag_in = [dram.tile([P, w], dt_mm, name=f"ag_in{i}")
                 for i, (_, w) in enumerate(chunks)]
        ag_out = [dram.tile([NCORES * P, w], dt_mm, addr_space="Shared",
                            name=f"ag_out{i}") for i, (_, w) in enumerate(chunks)]
        warm_in = dram.tile([1, 64], dt_mm, name="agw_in")
        warm_out = dram.tile([NCORES, 64], dt_mm, addr_space="Shared",
                             name="agw_out")

        singles = ctx.enter_context(tc.tile_pool(name="singles", bufs=1))

        # dummy AllGather to warm the collective path (queue/ring setup)
        # while the compute phases run, so the first real AG starts promptly
        nc.gpsimd.collective_compute(
            "AllGather", mybir.AluOpType.bypass, replica_groups=groups,
            ins=[warm_in[:].opt()], outs=[warm_out[:].opt()],
        )

        # weights: single fused DMA (wq|wk|wv|wo) on the ACT queue
        w4_sb = singles.tile([P, DC, 4, HC], dt_mm)
        nc.scalar.dma_start(
            out=w4_sb,
            in_=w4_d[:, :].rearrange("(c p) (g m) -> p c g m", p=P, g=4),
        )
        r2_sb = singles.tile([2, 3 * HC], dt_mm)      # [-csum | b] rows
        nc.scalar.dma_start(out=r2_sb, in_=r2_d[:, :])
        bo_t = singles.tile([P, 1], f32)
        nc.scalar.dma_start(out=bo_t, in_=bo_d[:].rearrange("(p o) -> p o", o=1))

        ident = singles.tile([P, P], dt_mm)
        make_identity(nc, ident)
        identf = singles.tile([P, P], f32)
        make_identity(nc, identf)
        warm_rhs = singles.tile([P, RC_W], dt_mm)
        nc.vector.memset(warm_rhs, 0.0)
        eps_t = singles.tile([P, 1], f32)
        nc.vector.memset(eps_t, LN_EPS)

        # long-lived activations
        xT = singles.tile([P, RT, DC, P], dt_mm)  # [dim%128, rt, dimchunk, row%128]
        mvs = singles.tile([P, RT, 2], f32)       # (mean, var) per row tile
        srb = singles.tile([P, RT, 2], f32)       # (std, rstd) per row tile
        r2rhs = singles.tile([2, N], dt_mm)       # [mu_row | std_row]
        qT = singles.tile([P, N], dt_mm)          # [2*64 qdims, rows]
        kT = singles.tile([P, N], dt_mm)
        vT = singles.tile([P, N], dt_mm)
        v_sb = singles.tile([P, RT, HL, DH + 1], dt_mm)  # [keys, rt, h, v|1]
        attn_h = [singles.tile([DH, N], dt_mm, name=f"attn_h{h}")
                  for h in range(HL)]
        outT = singles.tile([P, N], f32)

        nc.gpsimd.memset(v_sb[:, :, :, DH:], 1.0)  # ones column

        # ---- stage A: transposes + stats + QKV, fused LN ---------------------
        with (
            tc.tile_pool(name="xp", bufs=RT) as xp,
            tc.tile_pool(name="stat", bufs=4) as statp,
            tc.tile_pool(name="bcp", bufs=2) as bcp,
            tc.tile_pool(name="warmp", bufs=1, space="PSUM") as warmp,
            tc.tile_pool(name="stp", bufs=1, space="PSUM") as stp,
            tc.tile_pool(name="ptp", bufs=2, space="PSUM") as ptp,
            tc.tile_pool(name="mmp", bufs=3, space="PSUM") as mmp,
        ):
            # x-tile DMAs on the sync queue: everything downstream keys off
            # these, and the transposes/stats consume them at DMA pace.
            x_tiles = [xp.tile([P, D], dt_mm, tag="x", name=f"x{rt}")
                       for rt in range(RT)]
            for rt in range(RT):
                nc.sync.dma_start(out=x_tiles[rt],
                                  in_=x_d[rt * P:(rt + 1) * P, :])

            # short dependency-free matmul burst to ramp the PE p-state
            warm_ps = warmp.tile([P, 512], f32, tag="warm")
            for _ in range(8):
                nc.tensor.matmul(warm_ps, ident, warm_rhs,
                                 start=True, stop=True)

            # loop 1: per tile, XBAR-transpose raw x (ACT hwdge queue) and
            # bn stats (DVE).  No cross-engine round trips inside the loop,
            # so both streams flow at x-DMA pace.
            for rt in range(RT):
                nc.scalar.dma_start_transpose(
                    out=xT[:, rt], in_=x_tiles[rt][:],
                )
                st = statp.tile([P, 2, 6], f32, tag="st")
                for sg in range(2):
                    nc.vector.bn_stats(
                        out=st[:, sg, :],
                        in_=x_tiles[rt][:, sg * 512:(sg + 1) * 512],
                    )
                nc.vector.bn_aggr(out=mvs[:, rt, :], in_=st)

            # loop 2: per 4-tile block: finish stats (std/rstd), build the
            # mu/std/rstd rows via tiny PE transposes, then the QKV chains
            # with the rank-2 LN correction folded in.
            for g in range(RT // 4):
                g0, g1 = g * RC_W, (g + 1) * RC_W
                for t in range(4):
                    rt = 4 * g + t
                    nc.scalar.activation(
                        out=srb[:, rt, 0:1], in_=mvs[:, rt, 1:2],
                        func=AF.Sqrt, bias=eps_t, scale=1.0,
                    )
                    nc.vector.reciprocal(out=srb[:, rt, 1:2],
                                         in_=srb[:, rt, 0:1])
                statT = stp.tile([4, 3, P], f32, tag="statT")
                nc.tensor.transpose(statT[:, 0, :],
                                    mvs[:, 4 * g:4 * g + 4, 0:1], identf)
                nc.tensor.transpose(statT[:, 1, :],
                                    srb[:, 4 * g:4 * g + 4, 0:1], identf)
                nc.tensor.transpose(statT[:, 2, :],
                                    srb[:, 4 * g:4 * g + 4, 1:2], identf)
                musd = statp.tile([4, 2, P], dt_mm, tag="musd")
                with nc.allow_low_precision(reason="ln stat rows"):
                    nc.vector.tensor_copy(out=musd, in_=statT[:, 0:2, :])
                rstd4 = statp.tile([4, P], f32, tag="rstd4")
                nc.vector.tensor_copy(out=rstd4, in_=statT[:, 2, :])
                rstdrow = bcp.tile([1, RC_W], f32, tag="rrow")
                rstdB = bcp.tile([P, RC_W], f32, tag="rB")
                nc.gpsimd.dma_start(out=r2rhs[0:1, g0:g1], in_=musd[:, 0, :])
                nc.gpsimd.dma_start(out=r2rhs[1:2, g0:g1], in_=musd[:, 1, :])
                nc.gpsimd.dma_start(out=rstdrow[0:1, :], in_=rstd4[:, :])
                nc.gpsimd.partition_broadcast(
                    out_ap=rstdB[:, :], in_ap=rstdrow[0:1, :],
                )

                for p, dst in ((0, qT), (1, kT), (2, vT)):
                    pm = mmp.tile([P, RC_W], f32, tag="pm")
                    for kc in range(DC):
                        nc.tensor.matmul(
                            pm,
                            w4_sb[:, kc, p, :],
                            xT[:, 4 * g:4 * g + 4, kc, :],
                            start=(kc == 0), stop=False,
                        )
                    nc.tensor.matmul(
                        pm,
                        r2_sb[:, p * HC:(p + 1) * HC],
                        r2rhs[:, g0:g1],
                        start=False, stop=True,
                    )
                    with nc.allow_low_precision(reason="qkv bf16 wire"):
                        nc.vector.tensor_mul(
                            out=dst[:, g0:g1], in0=pm, in1=rstdB[:, :],
                        )
                # v^T -> v (row-major with ones column) for this block
                for t in range(4):
                    rt = 4 * g + t
                    pt = ptp.tile([P, P], dt_mm, tag="pt")
                    with nc.allow_low_precision(reason="transpose copy"):
                        nc.tensor.transpose(
                            pt, vT[:, rt * P:(rt + 1) * P], ident
                        )
                    nc.scalar.copy(
                        out=v_sb[:, rt, :, 0:DH],
                        in_=pt[:].rearrange("p (h d) -> p h d", h=HL),
                    )

        wo_sb = w4_sb[:, :, 3, :]

        # ---- stage D: attention, per-stage pipeline -------------------------
        # Per stage (512 rows): sim for both heads packs into disjoint PE row
        # groups into a 2-buffer PSUM pool (sim of kc+1 overlaps exp of kc on
        # ACT); attn@v consumes exp_t same-stage with a 2-chunk lag into a
        # double-buffered accumulator (po2 halves alternate per stage) so the
        # next stage's attn@v never waits on the previous stage's normalize.
        # The v stationary carries a leading ones column, so the softmax
        # denominator lands on PSUM partition 0 where the fast custom-DVE
        # reciprocal and the GpSimd partition broadcast operate. Each stage
        # ships both heads in ONE AllGather; its projection runs inside the
        # chain: po2 half idx%2 is free between norm_tail(idx+2) and
        # av(idx+4), and the gather DMA is pre-issued after norm_tail(idx+1).
        with (
            tc.tile_pool(name="expp", bufs=1) as expp,
            tc.tile_pool(name="rsum", bufs=6) as rsump,
            tc.tile_pool(name="simp", bufs=2, space="PSUM") as simp,
            tc.tile_pool(name="op", bufs=1, space="PSUM") as op,
            tc.tile_pool(name="agp", bufs=2) as agp,
        ):
            exp_t = expp.tile([P, RT, HL, RC_W], dt_mm, tag="exp")
            po2 = op.tile([P, 2, HL * RC_W], f32, tag="po")      # 4 banks

            def sim_exp(idx, kc):
                """Both heads' sim for one key chunk + exp evacuation."""
                r0, w = chunks[idx]
                ps = simp.tile([P, HL, RC_W], f32, tag="ps",
                               name=f"ps{idx}_{kc}")
                for h in range(HL):
                    nc.tensor.matmul(
                        ps[:, h, 0:w],
                        kT[h * DH:(h + 1) * DH, kc * P:(kc + 1) * P],
                        qT[h * DH:(h + 1) * DH, r0:r0 + w],
                        start=True, stop=True,
                    )
                if kc in DVE_EXP_KCS:
                    # Schraudolph: bf16 bits = int16(logit*128/ln2 + B)
                    nc.vector.tensor_scalar(
                        out=exp_t[:, kc, :, 0:w].bitcast(mybir.dt.int16),
                        in0=ps[:, :, 0:w],
                        scalar1=SIM_SCALE * _SCH_A, scalar2=_SCH_B,
                        op0=mybir.AluOpType.mult, op1=mybir.AluOpType.add,
                    )
                else:
                    nc.scalar.activation(
                        out=exp_t[:, kc, :, 0:w], in_=ps[:, :, 0:w],
                        func=AF.Exp, scale=SIM_SCALE,
                    )

            def av_pair(idx, kc):
                """attn@v for key chunk kc, both heads (alternating banks)."""
                r0, w = chunks[idx]
                for h in range(HL):
                    nc.tensor.matmul(
                        po2[0:DH + 1, idx % 2, h * RC_W:h * RC_W + w],
                        v_sb[:, kc, h, :],
                        exp_t[:, kc, h, 0:w],
                        start=(kc == 0), stop=(kc == RT - 1),
                    )

            def norm_tail(idx):
                """Normalize by softmax denominators, ship to the AG buffer."""
                r0, w = chunks[idx]
                dcs, rss, rbs = [], [], []
                for h in range(HL):
                    # denominator row: PSUM p64 -> SBUF p64 (DVE, same lane),
                    # then SBUF p64 -> SBUF p0 (gpsimd DMA, off the sync
                    # queue so projection gathers can't head-of-line block it)
                    d64 = rsump.tile([P, RC_W], f32, tag="d64",
                                     name=f"d64{idx}_{h}")
                    nc.vector.tensor_copy(
                        out=d64[DH:DH + 1, 0:w],
                        in_=po2[DH:DH + 1, idx % 2, h * RC_W:h * RC_W + w],
                    )
                    dc = rsump.tile([1, RC_W], f32, tag="dc",
                                    name=f"dc{idx}_{h}")
                    nc.gpsimd.dma_start(
                        out=dc[0:1, 0:w], in_=d64[DH:DH + 1, 0:w],
                    )
                    dcs.append(dc)
                for h in range(HL):
                    rs = rsump.tile([1, RC_W], f32, tag="rs",
                                    name=f"rs{idx}_{h}")
                    nc.vector.reciprocal_approx_fast(
                        out=rs[0:1, 0:w], in_=dcs[h][0:1, 0:w]
                    )
                    rss.append(rs)
                for h in range(HL):
                    rb = rsump.tile([DH, RC_W], f32, tag="rb",
                                    name=f"rb{idx}_{h}")
                    nc.gpsimd.partition_broadcast(
                        out_ap=rb[:, 0:w], in_ap=rss[h][0:1, 0:w],
                    )
                    rbs.append(rb)
                for h in range(HL):
                    with nc.allow_low_precision(reason="attn bf16 wire"):
                        nc.vector.tensor_mul(
                            out=attn_h[h][:, r0:r0 + w],
                            in0=po2[0:DH, idx % 2, h * RC_W:h * RC_W + w],
                            in1=rbs[h][:, 0:w],
                        )
                    nc.sync.dma_start(
                        out=ag_in[idx][h * DH:(h + 1) * DH, :],
                        in_=attn_h[h][:, r0:r0 + w],
                    )
                nc.gpsimd.collective_compute(
                    "AllGather",
                    mybir.AluOpType.bypass,
                    replica_groups=groups,
                    ins=[ag_in[idx][:].opt()],
                    outs=[ag_out[idx][:].opt()],
                )

            def gather_ag(idx):
                """Pre-issue the gather of this stage's AllGathered heads."""
                r0, w = chunks[idx]
                agt = agp.tile([P, DC, RC_W], dt_mm, tag="agt",
                               name=f"agt{idx}")
                src = ag_out[idx][:, :].rearrange("(c p) w -> p c w", p=P)
                nc.sync.dma_start(out=agt[:, :, 0:w], in_=src)
                return agt

            def proj_mm(idx, agt):
                """outT slice for this row chunk from the gathered heads."""
                r0, w = chunks[idx]
                pf = po2[:, idx % 2, 0:RC_W]
                for kc in range(DC):
                    nc.tensor.matmul(
                        pf[:, 0:w],
                        wo_sb[:, kc, :],
                        agt[:, kc, 0:w],
                        start=(kc == 0), stop=(kc == DC - 1),
                    )
                # evacuate on DVE, not ACT: an ACT evac here queues ahead of
                # later exp calls and stalls the attention stream
                nc.vector.tensor_scalar(
                    out=outT[:, r0:r0 + w], in0=pf[:, 0:w],
                    scalar1=bo_t, scalar2=None,
                    op0=mybir.AluOpType.add,
                )
                nc.sync.dma_start(
                    out=out_d[:, r0:r0 + w], in_=outT[:, r0:r0 + w]
                )

            # flat pipeline: attn@v trails sim/exp by 2 slots ACROSS stage
            # boundaries, so the PE stream never drains at a stage edge;
            # norm_tail(idx) is emitted as soon as its last attn@v is.
            # proj(idx) is emitted after norm_tail(idx+2) (AG idx completed
            # ~one stage earlier; its po2 half was just read by norm_tail
            # and is not written again until av(idx+4), so no PE stall).
            slots = [(idx, kc) for idx in range(S) for kc in range(RT)]
            agts = {}
            pending = []  # (ready_slot, proj_idx): 4-slot lag past the
            # norm_tail whose po2 reads the proj matmuls would WAR-stall on
            for i, (idx, kc) in enumerate(slots):
                if pending and i >= pending[0][0]:
                    pj = pending.pop(0)[1]
                    proj_mm(pj, agts.pop(pj))
                sim_exp(idx, kc)
                if i >= 2:
                    pidx, pkc = slots[i - 2]
                    av_pair(pidx, pkc)
                    if pkc == RT - 1:
                        norm_tail(pidx)
                        if pidx >= 1:
                            agts[pidx - 1] = gather_ag(pidx - 1)
                        if pidx >= 2:
                            pending.append((i + 4, pidx - 2))
            for pidx, pkc in slots[-2:]:
                av_pair(pidx, pkc)
            norm_tail(S - 1)
            agts[S - 2] = gather_ag(S - 2)
            for pj in [p for _, p in pending]:
                proj_mm(pj, agts.pop(pj))
            # S-2 first: its po2 half has no WAR against norm_tail(S-1)'s
            # reads, so it runs while those drain
            proj_mm(S - 2, agts.pop(S - 2))
            proj_mm(S - 3, agts.pop(S - 3))
            agts[S - 1] = gather_ag(S - 1)
            proj_mm(S - 1, agts.pop(S - 1))

    if not nc.is_finalized():
        nc.finalize()
    return nc


def _get_built():
    global _BUILT
    if _BUILT is None:
        _BUILT = _build()
    return _BUILT


def _shard_inputs(x, ln_scale, ln_bias, w_qkv, w_out, b_out):
    """Host-side sharding: slice per-head weight columns, fold LN params."""
    ln_scale = np.asarray(ln_scale, np.float32)
    ln_bias = np.asarray(ln_bias, np.float32)
    w_qkv = np.asarray(w_qkv, np.float32)
    w_out = np.asarray(w_out, np.float32)
    b_out = np.asarray(b_out, np.float32)

    w_np = {"f32": np.float32, "f32r": np.float32,
            "bf16": ml_dtypes.bfloat16}[MM_DT]
    x = np.ascontiguousarray(np.asarray(x, np.float32).astype(w_np))

    in_maps = []
    for ci in range(NCORES):
        c0 = ci * HC
        ws = []
        r2 = np.zeros((2, 3 * HC), np.float32)
        for pi, off in enumerate((0, HEADS * DH, 2 * HEADS * DH)):
            w = w_qkv[:, off + c0: off + c0 + HC]
            wp = ln_scale[:, None] * w
            ws.append(wp)
            r2[0, pi * HC:(pi + 1) * HC] = -wp.sum(axis=0)   # -csum
            r2[1, pi * HC:(pi + 1) * HC] = ln_bias @ w       # bias
        ws.append(w_out[:, c0:c0 + HC])
        in_maps.append({
            "x": x,
            "w4": np.ascontiguousarray(
                np.concatenate(ws, axis=1).astype(w_np)),
            "r2": np.ascontiguousarray(r2.astype(w_np)),
            "bo": np.ascontiguousarray(b_out[c0:c0 + HC].astype(np.float32)),
        })
    return in_maps


def kernel(x, ln_scale, ln_bias, w_qkv, w_out, b_out):
    from concourse.bass_utils import run_bass_kernel_spmd

    nc = _get_built()
    in_maps = _shard_inputs(x, ln_scale, ln_bias, w_qkv, w_out, b_out)
    res = run_bass_kernel_spmd(nc, in_maps, core_ids=list(range(NCORES)))
    shards = [res.results[ci]["out"] for ci in range(NCORES)]  # [128, 2048] each
    outT = np.concatenate(shards, axis=0)  # [1024, 2048]
    return np.ascontiguousarray(outT.T)


# revision 15
# speedup vs baseline: 1.3663x; 1.3663x over previous
"""Distributed Trainium2 kernel for pre-LN multi-head self-attention.

Reference computation (n=2048, d=1024, 16 heads x 64):
    xn  = LayerNorm(x) * ln_scale + ln_bias
    qkv = xn @ w_qkv ; split -> q,k,v [16, 2048, 64]
    sim = (q @ k^T) * d**-0.5 ; attn = softmax(sim)
    out = concat_heads(attn @ v) @ w_out + b_out

Sharding: 2 heads per core (tensor parallel). Each core:
  - projects its 2 heads' q/k/v, attention in transposed layout,
    ONE AllGather per row chunk, 128-col slice of the final projection.
Host assembles the 8 [128, 2048] outT shards into the [2048, 1024] output.

v3 changes vs v2 (224.6us):
  - RAW x is transposed by the DMA XBAR (dma_start_transpose, 14ns per
    16x128 tile) straight into xT sbuf: no PE transposes, no PSUM
    evacuations, and no LayerNorm->transpose serialization.  LN is
    algebraically folded into the QKV projection instead:
        qT = rstd_row * (W'^T x^T  - csum (x) mu_row + b (x) std_row)
    where W' = ln_scale-scaled weights, csum = column sums of W' and
    b = ln_bias @ W are computed on host and shipped as a tiny [2, 384]
    tensor; the two rank-1 terms are ONE extra [2]-contraction matmul
    accumulated into the QKV PSUM chain; the per-row rstd multiply is
    the PSUM evacuation (gpsimd tensor_mul with a broadcast rstd tile).
    mu/std/rstd rows are built from bn_stats via a tiny PE transpose of
    the per-tile stat columns.  Pre-attention drops ~71us -> ~30us.
  - output projections interleaved into the AllGather chain: po2 half
    idx%2 is free between norm_tail(idx+2) and av(idx+4), so proj(idx)
    runs there (its gather DMA pre-issued one stage earlier, right
    after norm_tail(idx+1)).  Only AG4+proj4 remain in the drain.
"""

import sys

import ml_dtypes
import numpy as np

for _p in ("/opt/trn_rl_repo", "/root/.axon_site/_ro/trn_rl_repo"):
    if _p not in sys.path:
        sys.path.append(_p)

N = 2048          # sequence length
D = 1024          # model dim
HEADS = 16
DH = 64
NCORES = 8
HL = HEADS // NCORES          # heads per core (2)
HC = HL * DH                  # head cols per core (128)
LN_EPS = 1e-6
SIM_SCALE = float(D) ** -0.5  # reference scales by input dim

P = 128
RT = N // P        # 16 row tiles
DC = D // P        # 8 dim chunks
RC_W = 512         # row-chunk width for attention/collective pipeline

MM_DT = "bf16"

# DVE-side fast-exp (Schraudolph bf16 bit trick) for these key chunks of
# every stage; offloads the ACT engine (the attention-phase bottleneck) at
# ~3% max per-element error on the affected attention weights.
DVE_EXP_KCS = (3, 7, 11, 15)
_SCH_A = 128.0 / float(np.log(2.0))          # per (sim*SIM_SCALE) logit
_SCH_B = 127.0 * 128.0 - 5.5                 # exponent bias - centering

_BUILT = None


def _build():
    """Build the SPMD Bass graph (same graph on all 8 cores)."""
    from contextlib import ExitStack

    import concourse.tile as tile
    from concourse import bacc, mybir
    from concourse.masks import make_identity

    f32 = mybir.dt.float32
    dt_mm = {"f32": f32, "f32r": mybir.dt.float32r,
             "bf16": mybir.dt.bfloat16}[MM_DT]
    AF = mybir.ActivationFunctionType

    nc = bacc.Bacc(None, num_devices=NCORES)

    x_d = nc.declare_dram_parameter("x", [N, D], dt_mm, isOutput=False)
    w4_d = nc.declare_dram_parameter("w4", [D, 4 * HC], dt_mm, isOutput=False)
    r2_d = nc.declare_dram_parameter("r2", [2, 3 * HC], dt_mm, isOutput=False)
    bo_d = nc.declare_dram_parameter("bo", [HC], f32, isOutput=False)
    out_d = nc.declare_dram_parameter("out", [HC, N], f32, isOutput=True)

    groups = [list(range(NCORES))]
    # the AllGather chain is saturated end-to-end, so the tail is set by the
    # LAST ops' durations: keep the early chunks big (fewer fixed costs)
    # and the trailing chunks small (short final ops)
    chunks = [(0, 512), (512, 512), (1024, 512), (1536, 256), (1792, 256)]
    S = len(chunks)

    with ExitStack() as ctx:
        tc = ctx.enter_context(tile.TileContext(nc))

        dram = ctx.enter_context(tc.tile_pool(name="dram", bufs=1, space="DRAM"))
        ag_in = [dram.tile([P, w], dt_mm, name=f"ag_in{i}")
                 for i, (_, w) in enumerate(chunks)]
        ag_out = [dram.tile([NCORES * P, w], dt_mm, addr_space="Shared",
                            name=f"ag_out{i}") for i, (_, w) in enumerate(chunks)]
        warm_in = dram.tile([1, 64], dt_mm, name="agw_in")
        warm_out = dram.tile([NCORES, 64], dt_mm, addr_space="Shared",
                             name="agw_out")

        singles = ctx.enter_context(tc.tile_pool(name="singles", bufs=1))

        # dummy AllGather to warm the collective path (queue/ring setup)
        # while the compute phases run, so the first real AG starts promptly
        nc.gpsimd.collective_compute(
            "AllGather", mybir.AluOpType.bypass, replica_groups=groups,
            ins=[warm_in[:].opt()], outs=[warm_out[:].opt()],
        )

        # weights: single fused DMA (wq|wk|wv|wo) on the ACT queue
        w4_sb = singles.tile([P, DC, 4, HC], dt_mm)
        nc.scalar.dma_start(
            out=w4_sb,
            in_=w4_d[:, :].rearrange("(c p) (g m) -> p c g m", p=P, g=4),
        )
        r2_sb = singles.tile([2, 3 * HC], dt_mm)      # [-csum | b] rows
        nc.scalar.dma_start(out=r2_sb, in_=r2_d[:, :])
        bo_t = singles.tile([P, 1], f32)
        nc.scalar.dma_start(out=bo_t, in_=bo_d[:].rearrange("(p o) -> p o", o=1))

        ident = singles.tile([P, P], dt_mm)
        make_identity(nc, ident)
        identf = singles.tile([P, P], f32)
        make_identity(nc, identf)
        warm_rhs = singles.tile([P, RC_W], dt_mm)
        nc.vector.memset(warm_rhs, 0.0)
        eps_t = singles.tile([P, 1], f32)
        nc.vector.memset(eps_t, LN_EPS)

        # long-lived activations
        xT = singles.tile([P, RT, DC, P], dt_mm)  # [dim%128, rt, dimchunk, row%128]
        mvs = singles.tile([P, RT, 2], f32)       # (mean, var) per row tile
        srb = singles.tile([P, RT, 2], f32)       # (std, rstd) per row tile
        r2rhs = singles.tile([2, N], dt_mm)       # [mu_row | std_row]
        qT = singles.tile([P, N], dt_mm)          # [2*64 qdims, rows]
        kT = singles.tile([P, N], dt_mm)
        vT = singles.tile([P, N], dt_mm)
        v_sb = singles.tile([P, RT, HL, DH + 1], dt_mm)  # [keys, rt, h, v|1]
        attn_h = [singles.tile([DH, N], dt_mm, name=f"attn_h{h}")
                  for h in range(HL)]
        outT = singles.tile([P, N], f32)

        nc.gpsimd.memset(v_sb[:, :, :, DH:], 1.0)  # ones column

        # ---- stage A: transposes + stats + QKV, fused LN ---------------------
        with (
            tc.tile_pool(name="xp", bufs=RT) as xp,
            tc.tile_pool(name="stat", bufs=4) as statp,
            tc.tile_pool(name="bcp", bufs=2) as bcp,
            tc.tile_pool(name="warmp", bufs=1, space="PSUM") as warmp,
            tc.tile_pool(name="stp", bufs=1, space="PSUM") as stp,
            tc.tile_pool(name="ptp", bufs=2, space="PSUM") as ptp,
            tc.tile_pool(name="mmp", bufs=3, space="PSUM") as mmp,
        ):
            # x-tile DMAs on the sync queue: everything downstream keys off
            # these, and the transposes/stats consume them at DMA pace.
            x_tiles = [xp.tile([P, D], dt_mm, tag="x", name=f"x{rt}")
                       for rt in range(RT)]
            for rt in range(RT):
                nc.sync.dma_start(out=x_tiles[rt],
                                  in_=x_d[rt * P:(rt + 1) * P, :])

            # short dependency-free matmul burst to ramp the PE p-state
            warm_ps = warmp.tile([P, 512], f32, tag="warm")
            for _ in range(8):
                nc.tensor.matmul(warm_ps, ident, warm_rhs,
                                 start=True, stop=True)

            # loop 1: per tile, PE-transpose RAW x (no LayerNorm dependency:
            # transposes start as soon as the tile lands) and bn stats (DVE).
            # All transpose evacuations ride ACT, which is otherwise idle
            # here, so DVE stays dedicated to the stats stream.
            for rt in range(RT):
                for g in range(2):
                    pt = ptp.tile([P, 512], dt_mm, tag="pt")
                    with nc.allow_low_precision(reason="transpose copy"):
                        for j in range(4):
                            dc = g * 4 + j
                            nc.tensor.transpose(
                                pt[:, j * P:(j + 1) * P],
                                x_tiles[rt][:, dc * P:(dc + 1) * P],
                                ident,
                            )
                    nc.scalar.copy(
                        out=xT[:, rt, g * 4:(g + 1) * 4, :],
                        in_=pt[:].rearrange("p (j q) -> p j q", j=4),
                    )
                st = statp.tile([P, 2, 6], f32, tag="st")
                for sg in range(2):
                    nc.vector.bn_stats(
                        out=st[:, sg, :],
                        in_=x_tiles[rt][:, sg * 512:(sg + 1) * 512],
                    )
                nc.vector.bn_aggr(out=mvs[:, rt, :], in_=st)

            # loop 2: per 4-tile block: finish stats (std/rstd), build the
            # mu/std/rstd rows via tiny PE transposes, then the QKV chains
            # with the rank-2 LN correction folded in.
            for g in range(RT // 4):
                g0, g1 = g * RC_W, (g + 1) * RC_W
                for t in range(4):
                    rt = 4 * g + t
                    nc.scalar.activation(
                        out=srb[:, rt, 0:1], in_=mvs[:, rt, 1:2],
                        func=AF.Sqrt, bias=eps_t, scale=1.0,
                    )
                    nc.vector.reciprocal(out=srb[:, rt, 1:2],
                                         in_=srb[:, rt, 0:1])
                statT = stp.tile([4, 3, P], f32, tag="statT")
                nc.tensor.transpose(statT[:, 0, :],
                                    mvs[:, 4 * g:4 * g + 4, 0:1], identf)
                nc.tensor.transpose(statT[:, 1, :],
                                    srb[:, 4 * g:4 * g + 4, 0:1], identf)
                nc.tensor.transpose(statT[:, 2, :],
                                    srb[:, 4 * g:4 * g + 4, 1:2], identf)
                musd = statp.tile([4, 2, P], dt_mm, tag="musd")
                with nc.allow_low_precision(reason="ln stat rows"):
                    nc.vector.tensor_copy(out=musd, in_=statT[:, 0:2, :])
                rstd4 = statp.tile([4, P], f32, tag="rstd4")
                nc.vector.tensor_copy(out=rstd4, in_=statT[:, 2, :])
                rstdrow = bcp.tile([1, RC_W], f32, tag="rrow")
                rstdB = bcp.tile([P, RC_W], f32, tag="rB")
                nc.gpsimd.dma_start(out=r2rhs[0:1, g0:g1], in_=musd[:, 0, :])
                nc.gpsimd.dma_start(out=r2rhs[1:2, g0:g1], in_=musd[:, 1, :])
                nc.gpsimd.dma_start(out=rstdrow[0:1, :], in_=rstd4[:, :])
                nc.gpsimd.partition_broadcast(
                    out_ap=rstdB[:, :], in_ap=rstdrow[0:1, :],
                )

                for p, dst in ((0, qT), (1, kT), (2, vT)):
                    pm = mmp.tile([P, RC_W], f32, tag="pm")
                    for kc in range(DC):
                        nc.tensor.matmul(
                            pm,
                            w4_sb[:, kc, p, :],
                            xT[:, 4 * g:4 * g + 4, kc, :],
                            start=(kc == 0), stop=False,
                        )
                    nc.tensor.matmul(
                        pm,
                        r2_sb[:, p * HC:(p + 1) * HC],
                        r2rhs[:, g0:g1],
                        start=False, stop=True,
                    )
                    with nc.allow_low_precision(reason="qkv bf16 wire"):
                        nc.vector.tensor_mul(
                            out=dst[:, g0:g1], in0=pm, in1=rstdB[:, :],
                        )
                # v^T -> v (row-major with ones column) for this block
                for t in range(4):
                    rt = 4 * g + t
                    pt = ptp.tile([P, 512], dt_mm, tag="pt")
                    with nc.allow_low_precision(reason="transpose copy"):
                        nc.tensor.transpose(
                            pt[:, 0:P], vT[:, rt * P:(rt + 1) * P], ident
                        )
                    nc.scalar.copy(
                        out=v_sb[:, rt, :, 0:DH],
                        in_=pt[:, 0:P].rearrange("p (h d) -> p h d", h=HL),
                    )

        wo_sb = w4_sb[:, :, 3, :]

        # ---- stage D: attention, per-stage pipeline -------------------------
        # Per stage (512 rows): sim for both heads packs into disjoint PE row
        # groups into a 2-buffer PSUM pool (sim of kc+1 overlaps exp of kc on
        # ACT); attn@v consumes exp_t same-stage with a 2-chunk lag into a
        # double-buffered accumulator (po2 halves alternate per stage) so the
        # next stage's attn@v never waits on the previous stage's normalize.
        # The v stationary carries a leading ones column, so the softmax
        # denominator lands on PSUM partition 0 where the fast custom-DVE
        # reciprocal and the GpSimd partition broadcast operate. Each stage
        # ships both heads in ONE AllGather; its projection runs inside the
        # chain: po2 half idx%2 is free between norm_tail(idx+2) and
        # av(idx+4), and the gather DMA is pre-issued after norm_tail(idx+1).
        with (
            tc.tile_pool(name="expp", bufs=1) as expp,
            tc.tile_pool(name="rsum", bufs=6) as rsump,
            tc.tile_pool(name="simp", bufs=2, space="PSUM") as simp,
            tc.tile_pool(name="op", bufs=1, space="PSUM") as op,
            tc.tile_pool(name="agp", bufs=2) as agp,
        ):
            exp_t = expp.tile([P, RT, HL, RC_W], dt_mm, tag="exp")
            po2 = op.tile([P, 2, HL * RC_W], f32, tag="po")      # 4 banks

            def sim_exp(idx, kc):
                """Both heads' sim for one key chunk + exp evacuation."""
                r0, w = chunks[idx]
                ps = simp.tile([P, HL, RC_W], f32, tag="ps",
                               name=f"ps{idx}_{kc}")
                for h in range(HL):
                    nc.tensor.matmul(
                        ps[:, h, 0:w],
                        kT[h * DH:(h + 1) * DH, kc * P:(kc + 1) * P],
                        qT[h * DH:(h + 1) * DH, r0:r0 + w],
                        start=True, stop=True,
                    )
                if kc in DVE_EXP_KCS:
                    # Schraudolph: bf16 bits = int16(logit*128/ln2 + B)
                    nc.vector.tensor_scalar(
                        out=exp_t[:, kc, :, 0:w].bitcast(mybir.dt.int16),
                        in0=ps[:, :, 0:w],
                        scalar1=SIM_SCALE * _SCH_A, scalar2=_SCH_B,
                        op0=mybir.AluOpType.mult, op1=mybir.AluOpType.add,
                    )
                else:
                    nc.scalar.activation(
                        out=exp_t[:, kc, :, 0:w], in_=ps[:, :, 0:w],
                        func=AF.Exp, scale=SIM_SCALE,
                    )

            def av_pair(idx, kc):
                """attn@v for key chunk kc, both heads (alternating banks)."""
                r0, w = chunks[idx]
                for h in range(HL):
                    nc.tensor.matmul(
                        po2[0:DH + 1, idx % 2, h * RC_W:h * RC_W + w],
                        v_sb[:, kc, h, :],
                        exp_t[:, kc, h, 0:w],
                        start=(kc == 0), stop=(kc == RT - 1),
                    )

            def norm_tail(idx):
                """Normalize by softmax denominators, ship to the AG buffer."""
                r0, w = chunks[idx]
                dcs, rss, rbs = [], [], []
                for h in range(HL):
                    # denominator row: PSUM p64 -> SBUF p64 (DVE, same lane),
                    # then SBUF p64 -> SBUF p0 (gpsimd DMA, off the sync
                    # queue so projection gathers can't head-of-line block it)
                    d64 = rsump.tile([P, RC_W], f32, tag="d64",
                                     name=f"d64{idx}_{h}")
                    nc.vector.tensor_copy(
                        out=d64[DH:DH + 1, 0:w],
                        in_=po2[DH:DH + 1, idx % 2, h * RC_W:h * RC_W + w],
                    )
                    dc = rsump.tile([1, RC_W], f32, tag="dc",
                                    name=f"dc{idx}_{h}")
                    nc.gpsimd.dma_start(
                        out=dc[0:1, 0:w], in_=d64[DH:DH + 1, 0:w],
                    )
                    dcs.append(dc)
                for h in range(HL):
                    rs = rsump.tile([1, RC_W], f32, tag="rs",
                                    name=f"rs{idx}_{h}")
                    nc.vector.reciprocal_approx_fast(
                        out=rs[0:1, 0:w], in_=dcs[h][0:1, 0:w]
                    )
                    rss.append(rs)
                for h in range(HL):
                    rb = rsump.tile([DH, RC_W], f32, tag="rb",
                                    name=f"rb{idx}_{h}")
                    nc.gpsimd.partition_broadcast(
                        out_ap=rb[:, 0:w], in_ap=rss[h][0:1, 0:w],
                    )
                    rbs.append(rb)
                for h in range(HL):
                    with nc.allow_low_precision(reason="attn bf16 wire"):
                        nc.vector.tensor_mul(
                            out=attn_h[h][:, r0:r0 + w],
                            in0=po2[0:DH, idx % 2, h * RC_W:h * RC_W + w],
                            in1=rbs[h][:, 0:w],
                        )
                    nc.sync.dma_start(
                        out=ag_in[idx][h * DH:(h + 1) * DH, :],
                        in_=attn_h[h][:, r0:r0 + w],
                    )
                nc.gpsimd.collective_compute(
                    "AllGather",
                    mybir.AluOpType.bypass,
                    replica_groups=groups,
                    ins=[ag_in[idx][:].opt()],
                    outs=[ag_out[idx][:].opt()],
                )

            def gather_ag(idx):
                """Pre-issue the gather of this stage's AllGathered heads."""
                r0, w = chunks[idx]
                agt = agp.tile([P, DC, RC_W], dt_mm, tag="agt",
                               name=f"agt{idx}")
                src = ag_out[idx][:, :].rearrange("(c p) w -> p c w", p=P)
                nc.sync.dma_start(out=agt[:, :, 0:w], in_=src)
                return agt

            def proj_mm(idx, agt):
                """outT slice for this row chunk from the gathered heads."""
                r0, w = chunks[idx]
                pf = po2[:, idx % 2, 0:RC_W]
                for kc in range(DC):
                    nc.tensor.matmul(
                        pf[:, 0:w],
                        wo_sb[:, kc, :],
                        agt[:, kc, 0:w],
                        start=(kc == 0), stop=(kc == DC - 1),
                    )
                # evacuate on DVE, not ACT: an ACT evac here queues ahead of
                # later exp calls and stalls the attention stream
                nc.vector.tensor_scalar(
                    out=outT[:, r0:r0 + w], in0=pf[:, 0:w],
                    scalar1=bo_t, scalar2=None,
                    op0=mybir.AluOpType.add,
                )
                nc.sync.dma_start(
                    out=out_d[:, r0:r0 + w], in_=outT[:, r0:r0 + w]
                )

            # flat pipeline: attn@v trails sim/exp by 2 slots ACROSS stage
            # boundaries, so the PE stream never drains at a stage edge;
            # norm_tail(idx) is emitted as soon as its last attn@v is.
            # proj(idx) is emitted after norm_tail(idx+2) (AG idx completed
            # ~one stage earlier; its po2 half was just read by norm_tail
            # and is not written again until av(idx+4), so no PE stall).
            slots = [(idx, kc) for idx in range(S) for kc in range(RT)]
            agts = {}
            pending = []  # (ready_slot, proj_idx): 4-slot lag past the
            # norm_tail whose po2 reads the proj matmuls would WAR-stall on
            for i, (idx, kc) in enumerate(slots):
                if pending and i >= pending[0][0]:
                    pj = pending.pop(0)[1]
                    proj_mm(pj, agts.pop(pj))
                sim_exp(idx, kc)
                if i >= 2:
                    pidx, pkc = slots[i - 2]
                    av_pair(pidx, pkc)
                    if pkc == RT - 1:
                        norm_tail(pidx)
                        if pidx >= 1:
                            agts[pidx - 1] = gather_ag(pidx - 1)
                        if pidx >= 2:
                            pending.append((i + 4, pidx - 2))
            for pidx, pkc in slots[-2:]:
                av_pair(pidx, pkc)
            norm_tail(S - 1)
            agts[S - 2] = gather_ag(S - 2)
            for pj in [p for _, p in pending]:
                proj_mm(pj, agts.pop(pj))
            # S-2 first: its po2 half has no WAR against norm_tail(S-1)'s
            # reads, so it runs while those drain
            proj_mm(S - 2, agts.pop(S - 2))
            proj_mm(S - 3, agts.pop(S - 3))
            agts[S - 1] = gather_ag(S - 1)
            proj_mm(S - 1, agts.pop(S - 1))

    if not nc.is_finalized():
        nc.finalize()
    return nc


def _get_built():
    global _BUILT
    if _BUILT is None:
        _BUILT = _build()
    return _BUILT


def _shard_inputs(x, ln_scale, ln_bias, w_qkv, w_out, b_out):
    """Host-side sharding: slice per-head weight columns, fold LN params."""
    ln_scale = np.asarray(ln_scale, np.float32)
    ln_bias = np.asarray(ln_bias, np.float32)
    w_qkv = np.asarray(w_qkv, np.float32)
    w_out = np.asarray(w_out, np.float32)
    b_out = np.asarray(b_out, np.float32)

    w_np = {"f32": np.float32, "f32r": np.float32,
            "bf16": ml_dtypes.bfloat16}[MM_DT]
    x = np.ascontiguousarray(np.asarray(x, np.float32).astype(w_np))

    in_maps = []
    for ci in range(NCORES):
        c0 = ci * HC
        ws = []
        r2 = np.zeros((2, 3 * HC), np.float32)
        for pi, off in enumerate((0, HEADS * DH, 2 * HEADS * DH)):
            w = w_qkv[:, off + c0: off + c0 + HC]
            wp = ln_scale[:, None] * w
            ws.append(wp)
            r2[0, pi * HC:(pi + 1) * HC] = -wp.sum(axis=0)   # -csum
            r2[1, pi * HC:(pi + 1) * HC] = ln_bias @ w       # bias
        ws.append(w_out[:, c0:c0 + HC])
        in_maps.append({
            "x": x,
            "w4": np.ascontiguousarray(
                np.concatenate(ws, axis=1).astype(w_np)),
            "r2": np.ascontiguousarray(r2.astype(w_np)),
            "bo": np.ascontiguousarray(b_out[c0:c0 + HC].astype(np.float32)),
        })
    return in_maps


def kernel(x, ln_scale, ln_bias, w_qkv, w_out, b_out):
    from concourse.bass_utils import run_bass_kernel_spmd

    nc = _get_built()
    in_maps = _shard_inputs(x, ln_scale, ln_bias, w_qkv, w_out, b_out)
    res = run_bass_kernel_spmd(nc, in_maps, core_ids=list(range(NCORES)))
    shards = [res.results[ci]["out"] for ci in range(NCORES)]  # [128, 2048] each
    outT = np.concatenate(shards, axis=0)  # [1024, 2048]
    return np.ascontiguousarray(outT.T)
